# revision 21
# baseline (speedup 1.0000x reference)
"""Trainium2 Bass kernel for nn_MixedIGAB (2-layer IGAB dense-transformer block).

Sharding: 8 cores = (batch b = core//4) x (H-slab s = core%4, 32 rows each).
Halo replication (up to +-6 rows, host-padded) avoids neighbor exchange; one
AllReduce per layer (per-batch groups of 4 cores) carries the channel gram
G = X^T X.

v2 attention: the per-head channel attention only needs G -- scores are
A = Wk^T G Wq (diag 32x32 blocks), q/k norms are diag(W^T G W), and the
projection folds into M = attn_bd^T Wproj so o@Wproj becomes a single GEMM
pass over v.  Softmax scale-invariance lets G ride fp16 at 2^-6 scale.

LN: stats + xln for the whole row range up front (keeps the ACT engine inside
one activation-table set per phase), ln_g folded into ff1 weights host-side.

Depthwise 3x3 convs run on four lanes (PE diag-matmul taps / DVE ts+tt /
ACT-mul+DVE-add / DVE-mul+Pool-add) to balance engine busy time.
"""
import sys, os

sys.path.insert(0, "/opt/trn_rl_repo")
if "/root/.axon_site" not in sys.path:
    sys.path.append("/root/.axon_site")

import numpy as np
import ml_dtypes

import concourse.bass as bass
import concourse.bacc as bacc
import concourse.mybir as mybir
from concourse import bass_utils
from concourse import tile

F32 = mybir.dt.float32
F16 = mybir.dt.float16
AF = mybir.ActivationFunctionType
ALU = mybir.AluOpType

B, C, H, W = 2, 256, 128, 128
HEADS, D = 8, 32
LYR, FC = 2, 1024
WG = W + 2          # guarded width 130
HALO = 6
HB = 32 + 2 * HALO  # 44 buffer rows
NB = HB * WG        # flat elems per channel (5720)
N_CORES = 8
GROUPS = [[0, 1, 2, 3], [4, 5, 6, 7]]
EPS = 1e-5
P = 128
GSC = 1.0 / 64.0    # gram fp16 scale (cancels in softmax)

# per-layer row extents in buffer coords [lo, hi)
EXT = [
    dict(rv=(0, 44), rq=(6, 38), rp1=(1, 43), ry=(2, 42), rdw=(3, 41)),
    dict(rv=(3, 41), rq=(6, 38), rp1=(4, 40), ry=(5, 39), rdw=(6, 38)),
]
FF_CH = 8           # ff chunk rows
POS_CH = 14         # DVE-lane pos-conv chunk rows
TAPS = [(dy, dx) for dy in (-1, 0, 1) for dx in (-1, 0, 1)]

# conv engine lanes: 'pe' (diag matmul taps), 'dve' (ts-mul + tt-add),
# 'act' (ACT Copy-scale mul + DVE tt-add). No 'pool' lane: concurrent
# GpSimd SBUF traffic starves the DVE (measured 6x slowdown on DVE ops).
FFDW_ENG = ("pe", "pe", "pe", "pe", "act", "dve", "dve", "dve")
POS1_ENG = ("act", "pe")   # per ct
POS2_ENG = ("act", "pe")

DBG = [s for s in os.environ.get("KDBG", "").split(",") if s]


def _ntiles(total, step):
    out, o = [], 0
    while o < total:
        n = min(step, total - o)
        out.append((o, n))
        o += n
    return out


def _r3(ap_flat):
    return ap_flat.rearrange("p (r w) -> p r w", w=WG)


def _tap_src(in_fl, ins_fl, t, a, n):
    dy, dx = TAPS[t]
    s0 = (a + dy) * WG
    if dx == 0:
        return in_fl[:, s0:s0 + n]
    if dx == -1:
        return ins_fl[:, s0:s0 + n]
    return ins_fl[:, s0 + 2:s0 + 2 + n]


def _conv9(nc, pool, tag, out_ap, in_fl, ins_fl, wsc, a, b, mul_eng="dve",
           add_eng="dve"):
    """9-tap 3x3 depthwise conv on guarded flat layout.

    mul_eng: 'dve' -> tensor_scalar (4x) | 'act' -> ACT Copy with scale
    add_eng: 'dve' -> DVE tensor_tensor (2x) | 'pool' -> GpSimd tensor_tensor
    out rows [a, b) local to in_fl; in_fl covers [a-1, b+1); ins_fl is in_fl
    shifted right by one (see _shift1). Returns the output AP."""
    n = (b - a) * WG

    def ts(t):
        pr = pool.tile([P, n], F16, tag=tag + "p", name=tag + "p", bufs=3)
        src = _tap_src(in_fl, ins_fl, t, a, n)
        if mul_eng == "act":
            nc.scalar.activation(pr[:], src, AF.Copy, scale=wsc[:, t:t + 1])
        else:
            nc.vector.tensor_scalar_mul(pr[:], src, wsc[:, t:t + 1])
        return pr

    def add(x, y, dst=None):
        if dst is None:
            dst = pool.tile([P, n], F16, tag=tag + "s", name=tag + "s",
                            bufs=2)[:]
        if add_eng == "pool":
            nc.gpsimd.tensor_tensor(dst, x, y, ALU.add)
        else:
            nc.vector.tensor_tensor(dst, x, y, ALU.add)
        return dst

    r = add(ts(0)[:], ts(1)[:])
    for t in range(2, 8):
        r = add(r, ts(t)[:])
    return add(r, ts(8)[:], dst=out_ap)


def _conv9_pe(nc, psum_ap, diag, in_fl, a, off, n):
    """9 diagonal-matmul taps accumulating into psum_ap [128, n]."""
    for t in range(9):
        dy, dx = TAPS[t]
        s0 = (a + dy) * WG + dx + off
        nc.tensor.matmul(psum_ap, diag[t][:], in_fl[:, s0:s0 + n],
                         start=(t == 0), stop=(t == 8))


def _shift1(nc, dst_fl, src_fl, n):
    nc.vector.tensor_copy(dst_fl[:, 1:n], src_fl[:, 0:n - 1])
    nc.any.memset(dst_fl[:, n:n + 2], 0.0)


def _mask_rows(nc, flat_ap, rmask, lo, hi):
    for r in list(range(lo, min(HALO, hi))) + list(range(max(HB - HALO, lo), hi)):
        nc.vector.tensor_scalar_mul(flat_ap[:, r * WG:(r + 1) * WG],
                                    flat_ap[:, r * WG:(r + 1) * WG],
                                    rmask[:, r:r + 1])


def _zero_guards(nc, flat_ap, a, b):
    t3 = _r3(flat_ap)
    nc.any.memset(t3[:, a:b, 0:1], 0.0)
    nc.any.memset(t3[:, a:b, WG - 1:WG], 0.0)


def build():
    nc = bacc.Bacc("TRN2", target_bir_lowering=False, debug=False,
                   num_devices=N_CORES)

    def din(name, shape, dt):
        return nc.dram_tensor(name, list(shape), dt, kind="ExternalInput")

    xh_d = din("xh", (2, P, NB), F16)
    il_d = din("il", (2, P, NB), F16)
    rm_d = din("rmask", (P, HB), F32)
    id_d = din("id128", (P, P), F16)
    wq_d = din("wq", (LYR, 2, P, 256), F16)
    wk_d = din("wk", (LYR, 2, P, 256), F16)
    wv_d = din("wv", (LYR, 2, 2, P, P), F16)
    wpr_d = din("wpr", (LYR, 2, 2, P, P), F16)
    f1t_d = din("f1t", (LYR, 2, 8, P, P), F16)
    f3t_d = din("f3t", (LYR, 8, 2, P, P), F16)
    posw_d = din("posw", (LYR, 2, 2, P, 9), F32)
    dww_d = din("dww", (LYR, 8, P, 9), F32)
    dwdg_d = din("dwdg", (LYR, 8, 9, P, P), F16)
    psdg_d = din("psdg", (LYR, 2, 2, 9, P, P), F16)
    b1_d = din("b1", (LYR, 8, P, 1), F32)
    bpj_d = din("bpj", (LYR, 2, P, 1), F32)
    resc_d = din("resc", (LYR, P, 2), F32)

    out_d = nc.dram_tensor("out", [2, P, 32, W], F32, kind="ExternalOutput")
    dbg_d = {}

    def dbg(name, src_ap, shape, dt=F16):
        if name not in DBG:
            return
        t = nc.dram_tensor("dbg_" + name, list(shape), dt, kind="ExternalOutput")
        dbg_d[name] = t
        nc.sync.dma_start(t.ap(), src_ap)

    with tile.TileContext(nc) as tc:
        with tc.tile_pool(name="persist", bufs=1) as pp, \
             tc.tile_pool(name="dstage", bufs=1, space="DRAM") as dp:

            xa = [pp.tile([P, NB], F16, tag=f"xa{i}", name=f"xa{i}") for i in range(2)]
            xb = [pp.tile([P, NB], F16, tag=f"xb{i}", name=f"xb{i}") for i in range(2)]
            ones_h = pp.tile([P, P], F16, tag="ones_h", name="ones_h")
            nc.any.memset(ones_h[:], 1.0)
            id_sb = pp.tile([P, P], F16, tag="id_sb", name="id_sb")
            nc.sync.dma_start(id_sb[:], id_d.ap())
            rmask = pp.tile([P, HB], F32, tag="rmask", name="rmask")
            nc.sync.dma_start(rmask[:], rm_d.ap())
            eps_t = pp.tile([P, 1], F32, tag="eps_t", name="eps_t")
            nc.any.memset(eps_t[:], EPS)
            cinv_t = pp.tile([P, 1], F32, tag="cinv_t", name="cinv_t")
            nc.any.memset(cinv_t[:], 1.0 / C)
            for i in range(2):
                for (o, n) in _ntiles(NB, 1430):
                    nc.sync.dma_start(xa[i][:, o:o + n],
                                      xh_d.ap()[i][:, o:o + n])

            cc_in = [dp.tile([256, 256], F32, tag=f"ccin{l}", name=f"ccin{l}") for l in range(LYR)]
            cc_out = [dp.tile([256, 256], F32, tag=f"ccout{l}", name=f"ccout{l}") for l in range(LYR)]

            for l in range(LYR):
                with tc.tile_pool(name=f"wp{l}", bufs=1) as wp:
                    def wt(dram_ap, shape, tag):
                        t = wp.tile(list(shape), dram_ap.dtype, tag=tag, name=tag)
                        nc.sync.dma_start(t[:], dram_ap)
                        return t

                    w = dict(
                        wq=[wt(wq_d.ap()[l, ct], [P, 256], f"wq{ct}")
                            for ct in range(2)],
                        wk=[wt(wk_d.ap()[l, ct], [P, 256], f"wk{ct}")
                            for ct in range(2)],
                        wv=[[wt(wv_d.ap()[l, kt, mt], [P, P], f"wv{kt}{mt}")
                             for mt in range(2)] for kt in range(2)],
                    )

                    def wload(phase, l=l, w=w, wt=wt):
                        if phase == "pos" and "posw" not in w:
                            w["posw"] = [[wt(posw_d.ap()[l, cv, ct], [P, 9], f"pw{cv}{ct}")
                                          for ct in range(2)] for cv in range(2)]
                            w["posdiag"] = {}
                            for cv, engs in ((0, POS1_ENG), (1, POS2_ENG)):
                                for ct in range(2):
                                    if engs[ct] in ("pe", "split"):
                                        w["posdiag"][(cv, ct)] = [
                                            wt(psdg_d.ap()[l, cv, ct, t], [P, P], f"pg{cv}{ct}{t}")
                                            for t in range(9)]
                            w["resc"] = wt(resc_d.ap()[l], [P, 2], "rsc")
                        if phase == "oy" and "wpr" not in w:
                            w["wpr"] = [[wt(wpr_d.ap()[l, kt, mt], [P, P], f"wpr{kt}{mt}")
                                         for mt in range(2)] for kt in range(2)]
                            w["bpj"] = [wt(bpj_d.ap()[l, ct], [P, 1], f"bpj{ct}")
                                        for ct in range(2)]
                        if phase == "ff" and "f1t" not in w:
                            w["f1t"] = [[wt(f1t_d.ap()[l, kt, mt], [P, P], f"f1t{kt}{mt}")
                                         for mt in range(8)] for kt in range(2)]
                            w["f3t"] = [[wt(f3t_d.ap()[l, kt, mt], [P, P], f"f3t{kt}{mt}")
                                         for mt in range(2)] for kt in range(8)]
                            w["dww"] = [wt(dww_d.ap()[l, mt], [P, 9], f"dw{mt}")
                                        if FFDW_ENG[mt] != "pe" else None
                                        for mt in range(8)]
                            w["dwdiag"] = [[wt(dwdg_d.ap()[l, mt, t], [P, P], f"dg{mt}{t}")
                                            for t in range(9)]
                                           if FFDW_ENG[mt] == "pe" else None
                                           for mt in range(8)]
                            w["b1"] = [wt(b1_d.ap()[l, mt], [P, 1], f"b1{mt}")
                                       for mt in range(8)]

                    _layer(nc, tc, l, w, wload, xa, xb, il_d, ones_h, id_sb,
                           eps_t, cinv_t, rmask,
                           cc_in[l], cc_out[l],
                           out_d if l == LYR - 1 else None, dbg)

    nc.compile()
    return nc, dbg_d


def _layer(nc, tc, l, w, wload, xa, xb, il_d, ones_h, id_sb, eps_t, cinv_t,
           rmask, cc_in, cc_out, out_d, dbg_raw):
    def dbg(name, src_ap, shape, dt=F16):
        dbg_raw(f"{name}.{l}", src_ap, shape, dt)

    ext = EXT[l]
    rv0, rv1 = ext["rv"]; rq0, rq1 = ext["rq"]
    rp10, rp11 = ext["rp1"]; ry0, ry1 = ext["ry"]; rdw0, rdw1 = ext["rdw"]

    with tc.tile_pool(name=f"mid{l}", bufs=1) as mp:
        v_sb = [mp.tile([P, NB + 2], F16, tag=f"v{i}", name=f"v{i}") for i in range(2)]
        for i in range(2):
            nc.any.memset(v_sb[i][:, NB:NB + 2], 0.0)
        p_acc = [mp.tile([P, NB], F16, tag=f"p{i}", name=f"p{i}") for i in range(2)]
        m_sb = [mp.tile([P, 256], F16, tag=f"m{i}", name=f"m{i}") for i in range(2)]

        # ============ phase 1: V and channel gram G = X^T X =================
        with tc.tile_pool(name=f"qs{l}", bufs=4) as qs, \
             tc.tile_pool(name=f"il{l}", bufs=1) as ilp, \
             tc.tile_pool(name=f"qps{l}", bufs=2, space="PSUM") as qps, \
             tc.tile_pool(name=f"gps{l}", bufs=1, space="PSUM") as gps:
            il_sb = [ilp.tile([P, NB], F16, tag=f"il{i}", name=f"il{i}")
                     for i in range(2)]
            for i in range(2):
                for (o, n) in _ntiles(NB, 1430):
                    nc.sync.dma_start(il_sb[i][:, o:o + n],
                                      il_d.ap()[i][:, o:o + n])

            # gram first: the AllReduce launches early and hides under V+pos1
            g_ps = [gps.tile([P, 256], F32, tag=f"g{mt}", name=f"g{mt}") for mt in range(2)]
            toks = _ntiles((rq1 - rq0) * WG, P)
            qbase = rq0 * WG
            ntk = len(toks)
            for ti, (o, m) in enumerate(toks):
                xt_ps = qps.tile([P, 2, P], F16, tag="xtps", name="xtps")
                for ct in range(2):
                    nc.tensor.transpose(
                        xt_ps[:m, ct, :],
                        xa[ct][:, qbase + o: qbase + o + m],
                        id_sb[:])
                xt_sb = qs.tile([P, 256], F16, tag="xtsb", name="xtsb")
                nc.vector.tensor_copy(xt_sb[:m, :],
                                      xt_ps[:m].rearrange("p a b -> p (a b)"))
                for mt in range(2):
                    nc.tensor.matmul(
                        g_ps[mt][:, :],
                        xt_sb[:m, P * mt: P * (mt + 1)],
                        xt_sb[:m, 0:256],
                        start=(ti == 0), stop=(ti == ntk - 1))

            for mt in range(2):
                g_st = qs.tile([P, 256], F32, tag="gst", name="gst")
                nc.vector.tensor_copy(g_st[:], g_ps[mt][:, :])
                nc.sync.dma_start(cc_in[P * mt: P * (mt + 1), :], g_st[:])
            nc.gpsimd.collective_compute(
                "AllReduce", ALU.add, replica_groups=GROUPS,
                ins=[cc_in.opt()], outs=[cc_out.opt()])

            vbase, vtot = rv0 * WG, (rv1 - rv0) * WG
            for mt in (1, 0):
                for (o, n) in _ntiles(vtot, 512):
                    v_ps = qps.tile([P, 512], F32, tag="vps", name="vps")
                    for kt in range(2):
                        nc.tensor.matmul(
                            v_ps[:, :n],
                            w["wv"][kt][mt][:],
                            xa[kt][:, vbase + o: vbase + o + n],
                            start=(kt == 0), stop=(kt == 1))
                    nc.vector.tensor_tensor(
                        v_sb[mt][:, vbase + o: vbase + o + n],
                        v_ps[:, :n],
                        il_sb[mt][:, vbase + o: vbase + o + n], ALU.mult)

        dbg("v", v_sb[0][:], (P, NB))

        wload("pos")
        # ============ phase 3: positional conv 1 (ct1 on PE first, then the
        # epilogue -- so its few ACT ops run before ct0's ACT-lane muls, and
        # the PE proj can start as soon as the allreduce lands) =============
        pg = [mp.tile([P, NB + 2], F16, tag=f"pg{i}", name=f"pg{i}") for i in range(2)]
        with tc.tile_pool(name=f"pos{l}", bufs=1) as cp, \
             tc.tile_pool(name=f"posa{l}", bufs=2) as ca, \
             tc.tile_pool(name=f"psps{l}", bufs=1, space="PSUM") as pps:
            vs = cp.tile([P, NB + 2], F16, tag="vs1", name="vs1")
            s1, e1 = rp10 * WG, rp11 * WG

            def pos1_ct(ct):
                eng = POS1_ENG[ct]
                if eng == "pe":
                    for (o, n) in _ntiles(e1 - s1 - 1, 512):
                        ps1 = pps.tile([P, 512], F32, tag="ps1", name="ps1")
                        _conv9_pe(nc, ps1[:, :n], w["posdiag"][(0, ct)],
                                  v_sb[ct][:], 0, s1 + 1 + o, n)
                        nc.scalar.activation(pg[ct][:, s1 + 1 + o: s1 + 1 + o + n],
                                             ps1[:, :n], AF.Gelu)
                else:
                    _shift1(nc, vs[:], v_sb[ct][:, 0:NB], NB)
                    for (co, cn) in _ntiles(rp11 - rp10, POS_CH):
                        a, b2 = rp10 + co, rp10 + co + cn
                        acc = _conv9(nc, ca, f"cv{ct}", None, v_sb[ct][:], vs[:],
                                     w["posw"][0][ct], a, b2,
                                     mul_eng=("act" if eng == "act" else "dve"),
                                     add_eng="dve")
                        nc.scalar.activation(
                            pg[ct][:, a * WG: b2 * WG], acc, AF.Gelu)
                _zero_guards(nc, pg[ct][:, 0:NB], rp10, rp11)
                _mask_rows(nc, pg[ct][:, 0:NB], rmask, rp10, rp11)
                nc.any.memset(pg[ct][:, NB:NB + 2], 0.0)

            pos1_ct(1)
            _phase2(nc, tc, l, w, wload, mp, m_sb, cc_out, id_sb, ones_h,
                    eps_t, dbg)
            pos1_ct(0)

        # ============ phase 3b: positional conv 2 (ct1-PE first) ============
        with tc.tile_pool(name=f"pos2{l}", bufs=1) as cp2, \
             tc.tile_pool(name=f"posb{l}", bufs=2) as cb, \
             tc.tile_pool(name=f"ps2ps{l}", bufs=2, space="PSUM") as pps2:
            for ct in (1, 0):
                eng = POS2_ENG[ct]
                if eng == "pe":
                    s2, e2 = ry0 * WG, ry1 * WG
                    for (o, n) in _ntiles(e2 - s2, 512):
                        ps2 = pps2.tile([P, 512], F32, tag="ps2", name="ps2")
                        _conv9_pe(nc, ps2[:, :n], w["posdiag"][(1, ct)],
                                  pg[ct][:], 0, s2 + o, n)
                        nc.scalar.activation(p_acc[ct][:, s2 + o: s2 + o + n],
                                             ps2[:, :n], AF.Copy)
                else:
                    pgs = cp2.tile([P, NB + 2], F16, tag=f"pgs{ct}",
                                   name=f"pgs{ct}")
                    _shift1(nc, pgs[:], pg[ct][:, 0:NB], NB)
                    for (co, cn) in _ntiles(ry1 - ry0, POS_CH):
                        a, b2 = ry0 + co, ry0 + co + cn
                        _conv9(nc, cb, f"c2{ct}",
                               p_acc[ct][:, a * WG: b2 * WG],
                               pg[ct][:], pgs[:],
                               w["posw"][1][ct], a, b2,
                               mul_eng=("act" if eng == "act" else "dve"),
                               add_eng="dve")

        dbg("p", p_acc[0][:], (P, NB))
        _phase4(nc, tc, l, w, xa, xb, v_sb, p_acc, m_sb, rmask, dbg)

    wload("ff")
    _phase5(nc, tc, l, w, xa, xb, ones_h, eps_t, cinv_t, out_d, dbg)


def _phase2(nc, tc, l, w, wload, mp, m_sb, cc_out, id_sb, ones_h, eps_t, dbg):
    with tc.tile_pool(name=f"att{l}", bufs=1) as ap, \
         tc.tile_pool(name=f"aps{l}", bufs=1, space="PSUM") as aps:
            g16 = [ap.tile([P, 256], F16, tag=f"g16{ct}", name=f"g16{ct}") for ct in range(2)]
            for ct in range(2):
                g_f = ap.tile([P, 256], F32, tag="gf", name="gf", bufs=2)
                nc.sync.dma_start(g_f[:], cc_out[P * ct: P * (ct + 1), :])
                nc.scalar.activation(g16[ct][:], g_f[:], AF.Copy,
                                     scale=GSC)

            # Bq = G @ Wq, Bk = G @ Wk  (fp16, channel-major halves)
            bq, bk = [], []
            for j, (wmat, blist) in enumerate(((w["wq"], bq), (w["wk"], bk))):
                for mt in range(2):
                    b_ps = aps.tile([P, 256], F32, tag="bps", name="bps")
                    for ct in range(2):
                        nc.tensor.matmul(b_ps[:],
                                         g16[ct][:, P * mt: P * (mt + 1)],
                                         wmat[ct][:],
                                         start=(ct == 0), stop=(ct == 1))
                    b_sb = ap.tile([P, 256], F16, tag=f"bsb{j}{mt}",
                                   name=f"bsb{j}{mt}")
                    nc.scalar.activation(b_sb[:], b_ps[:], AF.Copy)
                    blist.append(b_sb)

            # attention score blocks A_h = Wk_h^T (G Wq)_h  -> [128, 2, 128]
            a_ps = aps.tile([P, 256], F32, tag="aps", name="aps")
            for h in range(HEADS):
                hc, r = h // 4, h % 4
                for mt in range(2):
                    nc.tensor.matmul(
                        a_ps[32 * r: 32 * r + 32,
                             P * hc + 32 * r: P * hc + 32 * r + 32],
                        w["wk"][mt][:, 32 * h: 32 * h + 32],
                        bq[mt][:, 32 * h: 32 * h + 32],
                        start=(mt == 0), stop=(mt == 1),
                        tile_position=(0, 32 * r))

            # norms: sqq/sqk rows = diag(W^T G W) = colsum(W o (G W))
            sq_ps = aps.tile([P, 512], F32, tag="sqps", name="sqps")
            for j, (wmat, bl) in enumerate(((w["wq"], bq), (w["wk"], bk))):
                for mt in range(2):
                    e_t = ap.tile([P, 256], F16, tag="et", name="et", bufs=2)
                    nc.vector.tensor_tensor(e_t[:], wmat[mt][:], bl[mt][:],
                                            ALU.mult)
                    nc.tensor.matmul(sq_ps[0:1, 256 * j: 256 * j + 256],
                                     ones_h[:, 0:1], e_t[:],
                                     start=(mt == 0), stop=(mt == 1))
            rsq_row = ap.tile([P, 512], F16, tag="rsqr", name="rsqr")
            with nc.allow_low_precision(reason="attn norm factors"):
                nc.scalar.activation(rsq_row[0:1, :], sq_ps[0:1, :],
                                     AF.Abs_reciprocal_sqrt,
                                     bias=eps_t[0:1, :])

            # bc: broadcast rsq_q row over partitions
            bc_ps = aps.tile([P, 256], F32, tag="bcps", name="bcps")
            nc.tensor.matmul(bc_ps[:], ones_h[0:1, :], rsq_row[0:1, 0:256],
                             start=True, stop=True)
            bc_sb = ap.tile([P, 256], F32, tag="bcsb", name="bcsb")
            nc.vector.tensor_copy(bc_sb[:], bc_ps[:])
            # sqk column form [128, 2] via PE transpose of the rsq_k row
            # ([P, 2, 2] keeps each fp16 PSUM write 4-byte aligned)
            sqk_ps = aps.tile([P, 2, 2], F16, tag="skps", name="skps")
            for hc in range(2):
                nc.tensor.transpose(sqk_ps[:, hc, 0:1],
                                    rsq_row[0:1, 256 + P * hc: 256 + P * (hc + 1)],
                                    id_sb[0:1, 0:1])
            sqk_sb = ap.tile([P, 2], F32, tag="sksb", name="sksb")
            nc.vector.tensor_tensor(sqk_sb[:], sqk_ps[:, :, 0], w["resc"][:],
                                    ALU.mult)

            # z blocks + softmax -> att16 (block-diagonal, zeros elsewhere)
            att16 = ap.tile([P, 256], F16, tag="att16", name="att16")
            nc.any.memset(att16[:], 0.0)
            z_sb = ap.tile([P, 256], F32, tag="z", name="z")
            nmax = ap.tile([P, 8], F32, tag="nmax", name="nmax")
            ssum = ap.tile([P, 8], F32, tag="ssum", name="ssum")
            nc.any.memset(ssum[:], 1.0)
            esb = ap.tile([P, 256], F32, tag="esb", name="esb")
            for h in range(HEADS):
                hc, r = h // 4, h % 4
                po, fo = 32 * r, P * hc + 32 * r
                nc.vector.scalar_tensor_tensor(
                    z_sb[po:po + 32, fo:fo + 32],
                    a_ps[po:po + 32, fo:fo + 32],
                    sqk_sb[po:po + 32, hc:hc + 1],
                    bc_sb[po:po + 32, fo:fo + 32],
                    ALU.mult, ALU.mult)
                nc.vector.tensor_reduce(nmax[po:po + 32, h:h + 1],
                                        z_sb[po:po + 32, fo:fo + 32],
                                        mybir.AxisListType.X, ALU.max,
                                        negate=True)
                nc.scalar.activation(esb[po:po + 32, fo:fo + 32],
                                     z_sb[po:po + 32, fo:fo + 32], AF.Exp,
                                     bias=nmax[po:po + 32, h:h + 1],
                                     accum_out=ssum[po:po + 32, h:h + 1])
            nc.vector.reciprocal(ssum[:, 0:8], ssum[:, 0:8])
            for h in range(HEADS):
                hc, r = h // 4, h % 4
                po, fo = 32 * r, P * hc + 32 * r
                nc.vector.tensor_scalar_mul(att16[po:po + 32, fo:fo + 32],
                                            esb[po:po + 32, fo:fo + 32],
                                            ssum[po:po + 32, h:h + 1])
            dbg("att16", att16[:], (P, 256))

            wload("oy")
            # M = attn_bd^T Wproj  (so out_c = M^T v in one GEMM pass)
            for hc in range(2):
                m_ps = aps.tile([P, 256], F32, tag=f"mps{hc}", name=f"mps{hc}")
                for mt in range(2):
                    nc.tensor.matmul(m_ps[:, P * mt: P * (mt + 1)],
                                     att16[:, P * hc: P * (hc + 1)],
                                     w["wpr"][hc][mt][:],
                                     start=True, stop=True)
                nc.scalar.activation(m_sb[hc][:], m_ps[:], AF.Copy)


def _phase4(nc, tc, l, w, xa, xb, v_sb, p_acc, m_sb, rmask, dbg):
    ry0, ry1 = EXT[l]["ry"]
    # ============ phase 4: out_c = M^T v (+bpj +p), y = x + out_c + p ===
    with tc.tile_pool(name=f"oy{l}", bufs=3) as osp, \
         tc.tile_pool(name=f"oyps{l}", bufs=2, space="PSUM") as ops:
        ybase, ytot = ry0 * WG, (ry1 - ry0) * WG
        for (o, n) in _ntiles(ytot, 512):
            for mt in range(2):
                pr_ps = ops.tile([P, 512], F32, tag=f"prps{mt}", name=f"prps{mt}")
                for hc in range(2):
                    nc.tensor.matmul(pr_ps[:, :n],
                                     m_sb[hc][:, P * mt: P * (mt + 1)],
                                     v_sb[hc][:, ybase + o: ybase + o + n],
                                     start=(hc == 0), stop=(hc == 1))
                y1 = osp.tile([P, 512], F32, tag=f"y1{mt}", name=f"y1{mt}")
                nc.vector.scalar_tensor_tensor(
                    y1[:, :n], pr_ps[:, :n], w["bpj"][mt][:],
                    p_acc[mt][:, ybase + o: ybase + o + n],
                    ALU.add, ALU.add)
                nc.gpsimd.tensor_tensor(
                    xb[mt][:, ybase + o: ybase + o + n],
                    y1[:, :n],
                    xa[mt][:, ybase + o: ybase + o + n],
                    ALU.add)

    for ct in range(2):
        _mask_rows(nc, xb[ct][:], rmask, ry0, ry1)

    dbg("y", xb[0][:], (P, NB))


def _phase5(nc, tc, l, w, xa, xb, ones_h, eps_t, cinv_t, out_d, dbg):
    ry0, ry1 = EXT[l]["ry"]
    rdw0, rdw1 = EXT[l]["rdw"]
    ytot = (ry1 - ry0) * WG
    ybase = ry0 * WG
    # ============ phase 5a: LN stats + xln for the whole row range ==========
    with tc.tile_pool(name=f"lnp{l}", bufs=1) as lp:
        xln = [lp.tile([P, ytot], F16, tag=f"xln{ct}", name=f"xln{ct}") for ct in range(2)]
        rs16 = lp.tile([P, ytot], F16, tag="rs16", name="rs16")
        with tc.tile_pool(name=f"lns{l}", bufs=1) as ls, \
             tc.tile_pool(name=f"stps{l}", bufs=1, space="PSUM") as sps:
            for (o, n) in _ntiles(ytot, 512):
                ysq = [ls.tile([P, 512], F16, tag=f"ysq{ct}", name=f"ysq{ct}", bufs=2)
                       for ct in range(2)]
                for ct in range(2):
                    nc.vector.tensor_tensor(ysq[ct][:, :n],
                                            xb[ct][:, ybase + o: ybase + o + n],
                                            xb[ct][:, ybase + o: ybase + o + n],
                                            ALU.mult)
                ssum = sps.tile([P, 512], F32, tag="ssum", name="ssum", bufs=2)
                ssq = sps.tile([P, 512], F32, tag="ssq", name="ssq", bufs=2)
                for ct in range(2):
                    nc.tensor.matmul(ssum[:, :n], ones_h[:, :],
                                     xb[ct][:, ybase + o: ybase + o + n],
                                     start=(ct == 0), stop=(ct == 1))
                    nc.tensor.matmul(ssq[:, :n], ones_h[:, :],
                                     ysq[ct][:, :n],
                                     start=(ct == 0), stop=(ct == 1))
                mu2 = ls.tile([P, 512], F32, tag="mu2", name="mu2", bufs=2)
                nc.scalar.activation(mu2[:, :n], ssum[:, :n], AF.Square,
                                     scale=cinv_t[:])
                rs = ls.tile([P, 512], F32, tag="rs", name="rs", bufs=2)
                nc.vector.scalar_tensor_tensor(rs[:, :n], ssq[:, :n],
                                               1.0 / C, mu2[:, :n],
                                               ALU.mult, ALU.subtract)
                nc.scalar.activation(rs16[:, o:o + n], rs[:, :n],
                                     AF.Abs_reciprocal_sqrt, bias=eps_t[:])
                for ct in range(2):
                    d = ls.tile([P, 512], F16, tag=f"d{ct}", name=f"d{ct}", bufs=2)
                    nc.vector.scalar_tensor_tensor(
                        d[:, :n], ssum[:, :n], -1.0 / C,
                        xb[ct][:, ybase + o: ybase + o + n],
                        ALU.mult, ALU.add)
                    nc.vector.tensor_tensor(xln[ct][:, o:o + n], d[:, :n],
                                            rs16[:, o:o + n], ALU.mult)
        dbg("xln", xln[0][:], (P, ytot))

        # ============ phase 5b: FF (chunked over rows) ======================
        with tc.tile_pool(name=f"ff{l}", bufs=1) as fp, \
             tc.tile_pool(name=f"ffs{l}", bufs=1) as fs, \
             tc.tile_pool(name=f"ffps{l}", bufs=3, space="PSUM") as fps:
            for (co, cn) in _ntiles(ry1 - ry0, FF_CH):
                a, b = ry0 + co, ry0 + co + cn
                w0, w1 = max(a - 1, ry0), min(b + 1, ry1)
                c0, c1 = max(a, rdw0), min(b, rdw1)
                wlen = (w1 - w0) * WG
                xo = (w0 - ry0) * WG
                # --- ff1 + gelu -> t1 ---
                t1 = [fs.tile([P, wlen + 2], F16, tag=f"t1{mt}", name=f"t1{mt}") for mt in range(8)]
                t1s = [fs.tile([P, wlen + 2], F16, tag=f"t1s{mt}", name=f"t1s{mt}")
                       if FFDW_ENG[mt] != "pe" else None for mt in range(8)]
                for mt in range(8):
                    for (o, n) in _ntiles(wlen, 512):
                        f1_ps = fps.tile([P, 512], F32, tag="ffps", name="ffps")
                        for kt in range(2):
                            nc.tensor.matmul(f1_ps[:, :n], w["f1t"][kt][mt][:],
                                             xln[kt][:, xo + o: xo + o + n],
                                             start=(kt == 0), stop=(kt == 1))
                        nc.scalar.activation(t1[mt][:, o:o + n], f1_ps[:, :n],
                                             AF.Gelu, bias=w["b1"][mt][:])
                    _zero_guards(nc, t1[mt][:, 0:wlen], 0, w1 - w0)
                    nc.any.memset(t1[mt][:, wlen:wlen + 2], 0.0)
                    if FFDW_ENG[mt] != "pe":
                        _shift1(nc, t1s[mt][:], t1[mt][:, 0:wlen], wlen)
                # --- ffdw (4 lanes) + gelu -> t2 ---
                t2 = [fs.tile([P, (c1 - c0) * WG], F16, tag=f"t2{mt}", name=f"t2{mt}")
                      for mt in range(8)]
                for mt in range(8):
                    eng = FFDW_ENG[mt]
                    if eng == "pe":
                        base = (c0 - w0) * WG
                        for (o, n) in _ntiles((c1 - c0) * WG - 1, 512):
                            dw_ps = fps.tile([P, 512], F32, tag="dwps", name="dwps", bufs=2)
                            _conv9_pe(nc, dw_ps[:, :n], w["dwdiag"][mt],
                                      t1[mt][:], c0 - w0, o + 1, n)
                            nc.scalar.activation(t2[mt][:, 1 + o:1 + o + n],
                                                 dw_ps[:, :n], AF.Gelu)
                        nc.any.memset(t2[mt][:, 0:1], 0.0)
                    else:
                        acc = _conv9(nc, fs, f"dw{mt}", None, t1[mt][:, 0:wlen],
                                     t1s[mt][:], w["dww"][mt], c0 - w0, c1 - w0,
                                     mul_eng=("act" if eng == "act" else "dve"),
                                     add_eng=("pool" if eng == "pool" else "dve"))
                        nc.scalar.activation(t2[mt][:], acc, AF.Gelu)
                # --- ff3 + residual -> x2 (= xa), or final output ---
                for mt in range(2):
                    for (o, n) in _ntiles((c1 - c0) * WG, 512):
                        f3_ps = fps.tile([P, 512], F32, tag="f3ps", name="f3ps", bufs=2)
                        for kt in range(8):
                            nc.tensor.matmul(f3_ps[:, :n], w["f3t"][kt][mt][:],
                                             t2[kt][:, o:o + n],
                                             start=(kt == 0), stop=(kt == 7))
                        nc.vector.tensor_tensor(
                            xa[mt][:, c0 * WG + o: c0 * WG + o + n],
                            f3_ps[:, :n],
                            xb[mt][:, c0 * WG + o: c0 * WG + o + n], ALU.add)
                for mt in range(2):
                    _zero_guards(nc, xa[mt][:], c0, c1)

    if out_d is not None:
        with tc.tile_pool(name="outp", bufs=1) as op_:
            for ct in range(2):
                o32 = op_.tile([P, 32 * WG], F32, tag=f"o32{ct}", name=f"o32{ct}")
                nc.scalar.activation(o32[:], xa[ct][:, 6 * WG: 38 * WG], AF.Copy)
                nc.sync.dma_start(out_d.ap()[ct], _r3(o32[:])[:, :, 1:129])
    else:
        dbg("x2", xa[0][:], (P, NB))


# ======================== host side =========================================

_CACHE = {}


def _prep_shards(x, illu_fea, Wq, Wk, Wv, rescale, Wproj, bproj, pos1, pos2,
                 ln_g, ln_b, ff1, ffdw, ff3):
    f16 = ml_dtypes.float16 if hasattr(ml_dtypes, "float16") else np.float16

    def pad_spatial(t):  # (B,C,H,W) -> per-core [2, 128, HB, WG]
        out = []
        for core in range(N_CORES):
            bb, ss = core // 4, core % 4
            r0 = 32 * ss - HALO
            buf = np.zeros((C, HB, WG), np.float32)
            lo, hi = max(r0, 0), min(r0 + HB, H)
            buf[:, lo - r0: hi - r0, 1:129] = t[bb, :, lo:hi, :]
            out.append(buf.reshape(2, P, HB, WG))
        return out

    xs = pad_spatial(np.asarray(x, np.float32))
    ils = pad_spatial(np.asarray(illu_fea, np.float32))

    # channel-major attention weights [ct][c_local, m]
    wq = np.stack([Wq[l].reshape(2, P, 256) for l in range(LYR)])
    wk = np.stack([Wk[l].reshape(2, P, 256) for l in range(LYR)])
    wv = np.stack([Wv[l].reshape(2, P, 2, P).transpose(0, 2, 1, 3)
                   for l in range(LYR)])
    wpr = np.stack([Wproj[l].reshape(2, P, 2, P).transpose(0, 2, 1, 3)
                    for l in range(LYR)])
    # fold ln_g into ff1 (per input channel)
    f1 = np.stack([(ff1[l, :, :, 0, 0] * ln_g[l][None, :]).T
                   .reshape(2, P, 8, P).transpose(0, 2, 1, 3)
                   for l in range(LYR)])
    f3 = np.stack([ff3[l, :, :, 0, 0].T.reshape(8, P, 2, P).transpose(0, 2, 1, 3)
                   for l in range(LYR)])
    posw = np.stack([np.stack([p[l, :, 0].reshape(C, 9).reshape(2, P, 9)
                               for p in (pos1, pos2)]) for l in range(LYR)])
    dww = np.stack([ffdw[l, :, 0].reshape(FC, 9).reshape(8, P, 9)
                    for l in range(LYR)])
    dwdg = np.zeros((LYR, 8, 9, P, P), np.float32)
    psdg = np.zeros((LYR, 2, 2, 9, P, P), np.float32)
    ii = np.arange(P)
    for l in range(LYR):
        for mt in range(8):
            for t in range(9):
                dwdg[l, mt, t, ii, ii] = dww[l, mt, :, t]
        for cv in range(2):
            for ct in range(2):
                for t in range(9):
                    psdg[l, cv, ct, t, ii, ii] = posw[l, cv, ct, :, t]
    b1 = np.stack([(ff1[l, :, :, 0, 0] @ ln_b[l]).reshape(8, P, 1)
                   for l in range(LYR)])
    bpj = np.asarray(bproj, np.float32).reshape(LYR, 2, P, 1)
    # resc in sqk column layout: resc_col[p, hc] = rescale[head of 128*hc+p]
    resc = np.zeros((LYR, P, 2), np.float32)
    for l in range(LYR):
        for hc in range(2):
            for p in range(P):
                resc[l, p, hc] = rescale[l, (P * hc + p) // D, 0, 0]

    const = {
        "wq": wq.astype(f16), "wk": wk.astype(f16),
        "wv": wv.astype(f16), "wpr": wpr.astype(f16),
        "f1t": f1.astype(f16), "f3t": f3.astype(f16),
        "posw": posw.astype(np.float32), "dww": dww.astype(np.float32),
        "dwdg": dwdg.astype(f16), "psdg": psdg.astype(f16),
        "b1": b1.astype(np.float32), "bpj": bpj,
        "resc": resc.astype(np.float32),
        "id128": np.eye(P, dtype=np.float32).astype(f16),
    }
    in_maps = []
    for core in range(N_CORES):
        m = dict(const)
        ss = core % 4
        r0 = 32 * ss - HALO
        rmv = np.zeros((P, HB), np.float32)
        for r in range(HB):
            rmv[:, r] = 1.0 if 0 <= r0 + r < H else 0.0
        m["rmask"] = rmv.astype(np.float32)
        m["xh"] = xs[core].reshape(2, P, NB).astype(f16)
        m["il"] = ils[core].reshape(2, P, NB).astype(f16)
        in_maps.append(m)
    return in_maps


def _get_nc():
    if "nc" not in _CACHE:
        _CACHE["nc"], _CACHE["dbg"] = build()
    return _CACHE["nc"]


def run(in_maps, trace=False):
    nc = _get_nc()
    return bass_utils.run_bass_kernel_spmd(
        nc, in_maps, core_ids=list(range(N_CORES)), trace=trace)


def kernel(**inputs):
    in_maps = _prep_shards(**{k: np.asarray(v) for k, v in inputs.items()})
    res = run(in_maps)
    out = np.zeros((B, C, H, W), np.float32)
    for core in range(N_CORES):
        bb, ss = core // 4, core % 4
        o = res.results[core]["out"]  # [2, 128, 32, 128]
        out[bb, :, 32 * ss: 32 * ss + 32, :] = o.reshape(C, 32, W)
    return out


# revision 22
# speedup vs baseline: 1.1737x; 1.1737x over previous
"""Trainium2 Bass kernel for nn_MixedIGAB (2-layer IGAB dense-transformer block).

Sharding: 8 cores = (batch b = core//4) x (H-slab s = core%4, 32 rows each).
Halo replication (up to +-6 rows, host-padded) avoids neighbor exchange; one
AllReduce per layer (per-batch groups of 4 cores) carries the channel gram
G = X^T X.

v2 attention: the per-head channel attention only needs G -- scores are
A = Wk^T G Wq (diag 32x32 blocks), q/k norms are diag(W^T G W), and the
projection folds into M = attn_bd^T Wproj so o@Wproj becomes a single GEMM
pass over v.  Softmax scale-invariance lets G ride fp16 at 2^-6 scale.

LN: stats + xln for the whole row range up front (keeps the ACT engine inside
one activation-table set per phase), ln_g folded into ff1 weights host-side.

Depthwise 3x3 convs run on four lanes (PE diag-matmul taps / DVE ts+tt /
ACT-mul+DVE-add / DVE-mul+Pool-add) to balance engine busy time.
"""
import sys, os

sys.path.insert(0, "/opt/trn_rl_repo")
if "/root/.axon_site" not in sys.path:
    sys.path.append("/root/.axon_site")

import numpy as np
import ml_dtypes

import concourse.bass as bass
import concourse.bacc as bacc
import concourse.mybir as mybir
from concourse import bass_utils
from concourse import tile

F32 = mybir.dt.float32
F16 = mybir.dt.float16
AF = mybir.ActivationFunctionType
ALU = mybir.AluOpType

B, C, H, W = 2, 256, 128, 128
HEADS, D = 8, 32
LYR, FC = 2, 1024
WG = W + 2          # guarded width 130
HALO = 6
HB = 32 + 2 * HALO  # 44 buffer rows
NB = HB * WG        # flat elems per channel (5720)
N_CORES = 8
GROUPS = [[0, 1, 2, 3], [4, 5, 6, 7]]
EPS = 1e-5
P = 128
GSC = 1.0 / 64.0    # gram fp16 scale (cancels in softmax)

# per-layer row extents in buffer coords [lo, hi)
EXT = [
    dict(rv=(0, 44), rq=(6, 38), rp1=(1, 43), ry=(2, 42), rdw=(3, 41)),
    dict(rv=(3, 41), rq=(6, 38), rp1=(4, 40), ry=(5, 39), rdw=(6, 38)),
]
FF_CH = 8           # ff chunk rows
POS_CH = 14         # DVE-lane pos-conv chunk rows
TAPS = [(dy, dx) for dy in (-1, 0, 1) for dx in (-1, 0, 1)]

# conv engine lanes: 'pe' (diag matmul taps), 'dve' (ts-mul + tt-add),
# 'act' (ACT Copy-scale mul + DVE tt-add). No 'pool' lane: concurrent
# GpSimd SBUF traffic starves the DVE (measured 6x slowdown on DVE ops).
FFDW_ENG = ("pe", "pe", "pe", "pe", "act", "dve", "dve", "dve")
POS1_ENG = ("act", "pe")   # per ct
POS2_ENG = ("dve", "pe")

DBG = [s for s in os.environ.get("KDBG", "").split(",") if s]


def _ntiles(total, step):
    out, o = [], 0
    while o < total:
        n = min(step, total - o)
        out.append((o, n))
        o += n
    return out


def _r3(ap_flat):
    return ap_flat.rearrange("p (r w) -> p r w", w=WG)


def _tap_src(in_fl, ins_fl, t, a, n):
    dy, dx = TAPS[t]
    s0 = (a + dy) * WG
    if dx == 0:
        return in_fl[:, s0:s0 + n]
    if dx == -1:
        return ins_fl[:, s0:s0 + n]
    return ins_fl[:, s0 + 2:s0 + 2 + n]


def _conv9(nc, pool, tag, out_ap, in_fl, ins_fl, wsc, a, b, mul_eng="dve",
           add_eng="dve"):
    """9-tap 3x3 depthwise conv on guarded flat layout.

    mul_eng: 'dve' -> tensor_scalar (4x) | 'act' -> ACT Copy with scale
    add_eng: 'dve' -> DVE tensor_tensor (2x) | 'pool' -> GpSimd tensor_tensor
    out rows [a, b) local to in_fl; in_fl covers [a-1, b+1); ins_fl is in_fl
    shifted right by one (see _shift1). Returns the output AP."""
    n = (b - a) * WG

    def ts(t):
        pr = pool.tile([P, n], F16, tag=tag + "p", name=tag + "p", bufs=3)
        src = _tap_src(in_fl, ins_fl, t, a, n)
        if mul_eng == "act":
            nc.scalar.activation(pr[:], src, AF.Copy, scale=wsc[:, t:t + 1])
        else:
            nc.vector.tensor_scalar_mul(pr[:], src, wsc[:, t:t + 1])
        return pr

    def add(x, y, dst=None):
        if dst is None:
            dst = pool.tile([P, n], F16, tag=tag + "s", name=tag + "s",
                            bufs=2)[:]
        if add_eng == "pool":
            nc.gpsimd.tensor_tensor(dst, x, y, ALU.add)
        else:
            nc.vector.tensor_tensor(dst, x, y, ALU.add)
        return dst

    r = add(ts(0)[:], ts(1)[:])
    for t in range(2, 8):
        r = add(r, ts(t)[:])
    return add(r, ts(8)[:], dst=out_ap)


def _conv9_pe(nc, psum_ap, diag, in_fl, a, off, n):
    """9 diagonal-matmul taps accumulating into psum_ap [128, n]."""
    for t in range(9):
        dy, dx = TAPS[t]
        s0 = (a + dy) * WG + dx + off
        nc.tensor.matmul(psum_ap, diag[t][:], in_fl[:, s0:s0 + n],
                         start=(t == 0), stop=(t == 8))


def _shift1(nc, dst_fl, src_fl, n):
    nc.vector.tensor_copy(dst_fl[:, 1:n], src_fl[:, 0:n - 1])
    nc.any.memset(dst_fl[:, n:n + 2], 0.0)


def _mask_rows(nc, flat_ap, rmask, lo, hi):
    for r in list(range(lo, min(HALO, hi))) + list(range(max(HB - HALO, lo), hi)):
        nc.vector.tensor_scalar_mul(flat_ap[:, r * WG:(r + 1) * WG],
                                    flat_ap[:, r * WG:(r + 1) * WG],
                                    rmask[:, r:r + 1])


def _zero_guards(nc, flat_ap, a, b):
    t3 = _r3(flat_ap)
    nc.any.memset(t3[:, a:b, 0:1], 0.0)
    nc.any.memset(t3[:, a:b, WG - 1:WG], 0.0)


def build():
    nc = bacc.Bacc("TRN2", target_bir_lowering=False, debug=False,
                   num_devices=N_CORES)

    def din(name, shape, dt):
        return nc.dram_tensor(name, list(shape), dt, kind="ExternalInput")

    xh_d = din("xh", (2, P, NB), F16)
    il_d = din("il", (2, P, NB), F16)
    rm_d = din("rmask", (P, HB), F32)
    id_d = din("id128", (P, P), F16)
    wq_d = din("wq", (LYR, 2, P, 256), F16)
    wk_d = din("wk", (LYR, 2, P, 256), F16)
    wv_d = din("wv", (LYR, 2, 2, P, P), F16)
    wpr_d = din("wpr", (LYR, 2, 2, P, P), F16)
    f1t_d = din("f1t", (LYR, 2, 8, P, P), F16)
    f3t_d = din("f3t", (LYR, 8, 2, P, P), F16)
    posw_d = din("posw", (LYR, 2, 2, P, 9), F32)
    dww_d = din("dww", (LYR, 8, P, 9), F32)
    dwdg_d = din("dwdg", (LYR, 8, 9, P, P), F16)
    psdg_d = din("psdg", (LYR, 2, 2, 9, P, P), F16)
    b1_d = din("b1", (LYR, 8, P, 1), F32)
    bpj_d = din("bpj", (LYR, 2, P, 1), F32)
    resc_d = din("resc", (LYR, P, 2), F32)

    out_d = nc.dram_tensor("out", [2, P, 32, W], F32, kind="ExternalOutput")
    dbg_d = {}

    def dbg(name, src_ap, shape, dt=F16):
        if name not in DBG:
            return
        t = nc.dram_tensor("dbg_" + name, list(shape), dt, kind="ExternalOutput")
        dbg_d[name] = t
        nc.sync.dma_start(t.ap(), src_ap)

    with tile.TileContext(nc) as tc:
        with tc.tile_pool(name="persist", bufs=1) as pp, \
             tc.tile_pool(name="dstage", bufs=1, space="DRAM") as dp:

            xa = [pp.tile([P, NB], F16, tag=f"xa{i}", name=f"xa{i}") for i in range(2)]
            xb = [pp.tile([P, NB], F16, tag=f"xb{i}", name=f"xb{i}") for i in range(2)]
            ones_h = pp.tile([P, P], F16, tag="ones_h", name="ones_h")
            nc.any.memset(ones_h[:], 1.0)
            id_sb = pp.tile([P, P], F16, tag="id_sb", name="id_sb")
            nc.sync.dma_start(id_sb[:], id_d.ap())
            rmask = pp.tile([P, HB], F32, tag="rmask", name="rmask")
            nc.sync.dma_start(rmask[:], rm_d.ap())
            eps_t = pp.tile([P, 1], F32, tag="eps_t", name="eps_t")
            nc.any.memset(eps_t[:], EPS)
            cinv_t = pp.tile([P, 1], F32, tag="cinv_t", name="cinv_t")
            nc.any.memset(cinv_t[:], 1.0 / C)
            for i in range(2):
                for (o, n) in _ntiles(NB, 1430):
                    nc.sync.dma_start(xa[i][:, o:o + n],
                                      xh_d.ap()[i][:, o:o + n])

            cc_in = [dp.tile([256, 256], F32, tag=f"ccin{l}", name=f"ccin{l}") for l in range(LYR)]
            cc_out = [dp.tile([256, 256], F32, tag=f"ccout{l}", name=f"ccout{l}") for l in range(LYR)]

            for l in range(LYR):
                with tc.tile_pool(name=f"wp{l}", bufs=1) as wp:
                    def wt(dram_ap, shape, tag):
                        t = wp.tile(list(shape), dram_ap.dtype, tag=tag, name=tag)
                        nc.sync.dma_start(t[:], dram_ap)
                        return t

                    w = dict(
                        wq=[wt(wq_d.ap()[l, ct], [P, 256], f"wq{ct}")
                            for ct in range(2)],
                        wk=[wt(wk_d.ap()[l, ct], [P, 256], f"wk{ct}")
                            for ct in range(2)],
                        wv=[[wt(wv_d.ap()[l, kt, mt], [P, P], f"wv{kt}{mt}")
                             for mt in range(2)] for kt in range(2)],
                    )

                    def wload(phase, l=l, w=w, wt=wt):
                        if phase == "pos" and "posw" not in w:
                            w["posw"] = [[wt(posw_d.ap()[l, cv, ct], [P, 9], f"pw{cv}{ct}")
                                          for ct in range(2)] for cv in range(2)]
                            w["posdiag"] = {}
                            for cv, engs in ((0, POS1_ENG), (1, POS2_ENG)):
                                for ct in range(2):
                                    if engs[ct] in ("pe", "split"):
                                        w["posdiag"][(cv, ct)] = [
                                            wt(psdg_d.ap()[l, cv, ct, t], [P, P], f"pg{cv}{ct}{t}")
                                            for t in range(9)]
                            w["resc"] = wt(resc_d.ap()[l], [P, 2], "rsc")
                        if phase == "oy" and "wpr" not in w:
                            w["wpr"] = [[wt(wpr_d.ap()[l, kt, mt], [P, P], f"wpr{kt}{mt}")
                                         for mt in range(2)] for kt in range(2)]
                            w["bpj"] = [wt(bpj_d.ap()[l, ct], [P, 1], f"bpj{ct}")
                                        for ct in range(2)]
                        if phase == "ff" and "f1t" not in w:
                            w["f1t"] = [[wt(f1t_d.ap()[l, kt, mt], [P, P], f"f1t{kt}{mt}")
                                         for mt in range(8)] for kt in range(2)]
                            w["f3t"] = [[wt(f3t_d.ap()[l, kt, mt], [P, P], f"f3t{kt}{mt}")
                                         for mt in range(2)] for kt in range(8)]
                            w["dww"] = [wt(dww_d.ap()[l, mt], [P, 9], f"dw{mt}")
                                        if FFDW_ENG[mt] != "pe" else None
                                        for mt in range(8)]
                            w["dwdiag"] = [[wt(dwdg_d.ap()[l, mt, t], [P, P], f"dg{mt}{t}")
                                            for t in range(9)]
                                           if FFDW_ENG[mt] == "pe" else None
                                           for mt in range(8)]
                            w["b1"] = [wt(b1_d.ap()[l, mt], [P, 1], f"b1{mt}")
                                       for mt in range(8)]

                    _layer(nc, tc, l, w, wload, xa, xb, il_d, ones_h, id_sb,
                           eps_t, cinv_t, rmask,
                           cc_in[l], cc_out[l],
                           out_d if l == LYR - 1 else None, dbg)

    nc.compile()
    return nc, dbg_d


def _layer(nc, tc, l, w, wload, xa, xb, il_d, ones_h, id_sb, eps_t, cinv_t,
           rmask, cc_in, cc_out, out_d, dbg_raw):
    def dbg(name, src_ap, shape, dt=F16):
        dbg_raw(f"{name}.{l}", src_ap, shape, dt)

    ext = EXT[l]
    rv0, rv1 = ext["rv"]; rq0, rq1 = ext["rq"]
    rp10, rp11 = ext["rp1"]; ry0, ry1 = ext["ry"]; rdw0, rdw1 = ext["rdw"]

    with tc.tile_pool(name=f"mid{l}", bufs=1) as mp:
        v_sb = [mp.tile([P, NB + 2], F16, tag=f"v{i}", name=f"v{i}") for i in range(2)]
        for i in range(2):
            nc.any.memset(v_sb[i][:, NB:NB + 2], 0.0)
        p_acc = [mp.tile([P, NB], F16, tag=f"p{i}", name=f"p{i}") for i in range(2)]
        m_sb = [mp.tile([P, 256], F16, tag=f"m{i}", name=f"m{i}") for i in range(2)]

        # ============ phase 1: V and channel gram G = X^T X =================
        with tc.tile_pool(name=f"qs{l}", bufs=4) as qs, \
             tc.tile_pool(name=f"il{l}", bufs=1) as ilp, \
             tc.tile_pool(name=f"qps{l}", bufs=2, space="PSUM") as qps, \
             tc.tile_pool(name=f"gps{l}", bufs=1, space="PSUM") as gps:
            il_sb = [ilp.tile([P, NB], F16, tag=f"il{i}", name=f"il{i}")
                     for i in range(2)]
            for i in range(2):
                for (o, n) in _ntiles(NB, 1430):
                    nc.sync.dma_start(il_sb[i][:, o:o + n],
                                      il_d.ap()[i][:, o:o + n])

            # gram first: the AllReduce launches early and hides under V+pos1
            g_ps = [gps.tile([P, 256], F32, tag=f"g{mt}", name=f"g{mt}") for mt in range(2)]
            toks = _ntiles((rq1 - rq0) * WG, P)
            qbase = rq0 * WG
            ntk = len(toks)
            for ti, (o, m) in enumerate(toks):
                xt_ps = qps.tile([P, 2, P], F16, tag="xtps", name="xtps")
                for ct in range(2):
                    nc.tensor.transpose(
                        xt_ps[:m, ct, :],
                        xa[ct][:, qbase + o: qbase + o + m],
                        id_sb[:])
                xt_sb = qs.tile([P, 256], F16, tag="xtsb", name="xtsb")
                nc.vector.tensor_copy(xt_sb[:m, :],
                                      xt_ps[:m].rearrange("p a b -> p (a b)"))
                for mt in range(2):
                    nc.tensor.matmul(
                        g_ps[mt][:, :],
                        xt_sb[:m, P * mt: P * (mt + 1)],
                        xt_sb[:m, 0:256],
                        start=(ti == 0), stop=(ti == ntk - 1))

            for mt in range(2):
                g_st = qs.tile([P, 256], F32, tag="gst", name="gst")
                nc.vector.tensor_copy(g_st[:], g_ps[mt][:, :])
                nc.sync.dma_start(cc_in[P * mt: P * (mt + 1), :], g_st[:])
            nc.gpsimd.collective_compute(
                "AllReduce", ALU.add, replica_groups=GROUPS,
                ins=[cc_in.opt()], outs=[cc_out.opt()])

            vbase, vtot = rv0 * WG, (rv1 - rv0) * WG
            for mt in (1, 0):
                for (o, n) in _ntiles(vtot, 512):
                    v_ps = qps.tile([P, 512], F32, tag="vps", name="vps")
                    for kt in range(2):
                        nc.tensor.matmul(
                            v_ps[:, :n],
                            w["wv"][kt][mt][:],
                            xa[kt][:, vbase + o: vbase + o + n],
                            start=(kt == 0), stop=(kt == 1))
                    nc.vector.tensor_tensor(
                        v_sb[mt][:, vbase + o: vbase + o + n],
                        v_ps[:, :n],
                        il_sb[mt][:, vbase + o: vbase + o + n], ALU.mult)

        dbg("v", v_sb[0][:], (P, NB))

        wload("pos")
        # ============ phase 3: positional conv 1 (ct1 on PE first, then the
        # epilogue -- so its few ACT ops run before ct0's ACT-lane muls, and
        # the PE proj can start as soon as the allreduce lands) =============
        pg = [mp.tile([P, NB + 2], F16, tag=f"pg{i}", name=f"pg{i}") for i in range(2)]
        with tc.tile_pool(name=f"pos{l}", bufs=1) as cp, \
             tc.tile_pool(name=f"posa{l}", bufs=2) as ca, \
             tc.tile_pool(name=f"psps{l}", bufs=1, space="PSUM") as pps:
            vs = cp.tile([P, NB + 2], F16, tag="vs1", name="vs1")
            s1, e1 = rp10 * WG, rp11 * WG

            def pos1_ct(ct):
                eng = POS1_ENG[ct]
                if eng == "pe":
                    for (o, n) in _ntiles(e1 - s1 - 1, 512):
                        ps1 = pps.tile([P, 512], F32, tag="ps1", name="ps1")
                        _conv9_pe(nc, ps1[:, :n], w["posdiag"][(0, ct)],
                                  v_sb[ct][:], 0, s1 + 1 + o, n)
                        nc.scalar.activation(pg[ct][:, s1 + 1 + o: s1 + 1 + o + n],
                                             ps1[:, :n], AF.Gelu)
                else:
                    _shift1(nc, vs[:], v_sb[ct][:, 0:NB], NB)
                    for (co, cn) in _ntiles(rp11 - rp10, POS_CH):
                        a, b2 = rp10 + co, rp10 + co + cn
                        acc = _conv9(nc, ca, f"cv{ct}", None, v_sb[ct][:], vs[:],
                                     w["posw"][0][ct], a, b2,
                                     mul_eng=("act" if eng == "act" else "dve"),
                                     add_eng="dve")
                        nc.scalar.activation(
                            pg[ct][:, a * WG: b2 * WG], acc, AF.Gelu)
                _zero_guards(nc, pg[ct][:, 0:NB], rp10, rp11)
                _mask_rows(nc, pg[ct][:, 0:NB], rmask, rp10, rp11)
                nc.any.memset(pg[ct][:, NB:NB + 2], 0.0)

            pos1_ct(1)
            _phase2(nc, tc, l, w, wload, mp, m_sb, cc_out, id_sb, ones_h,
                    eps_t, dbg)
            pos1_ct(0)

        # ============ phase 3b: positional conv 2 (ct1-PE first) ============
        with tc.tile_pool(name=f"pos2{l}", bufs=1) as cp2, \
             tc.tile_pool(name=f"posb{l}", bufs=2) as cb, \
             tc.tile_pool(name=f"ps2ps{l}", bufs=2, space="PSUM") as pps2:
            for ct in (1, 0):
                eng = POS2_ENG[ct]
                if eng == "pe":
                    s2, e2 = ry0 * WG, ry1 * WG
                    for (o, n) in _ntiles(e2 - s2, 512):
                        ps2 = pps2.tile([P, 512], F32, tag="ps2", name="ps2")
                        _conv9_pe(nc, ps2[:, :n], w["posdiag"][(1, ct)],
                                  pg[ct][:], 0, s2 + o, n)
                        nc.scalar.activation(p_acc[ct][:, s2 + o: s2 + o + n],
                                             ps2[:, :n], AF.Copy)
                else:
                    pgs = cp2.tile([P, NB + 2], F16, tag=f"pgs{ct}",
                                   name=f"pgs{ct}")
                    _shift1(nc, pgs[:], pg[ct][:, 0:NB], NB)
                    for (co, cn) in _ntiles(ry1 - ry0, POS_CH):
                        a, b2 = ry0 + co, ry0 + co + cn
                        _conv9(nc, cb, f"c2{ct}",
                               p_acc[ct][:, a * WG: b2 * WG],
                               pg[ct][:], pgs[:],
                               w["posw"][1][ct], a, b2,
                               mul_eng=("act" if eng == "act" else "dve"),
                               add_eng="dve")

        dbg("p", p_acc[0][:], (P, NB))
        _phase4(nc, tc, l, w, xa, xb, v_sb, p_acc, m_sb, rmask, dbg)

    wload("ff")
    _phase5(nc, tc, l, w, xa, xb, ones_h, eps_t, cinv_t, out_d, dbg)


def _phase2(nc, tc, l, w, wload, mp, m_sb, cc_out, id_sb, ones_h, eps_t, dbg):
    with tc.tile_pool(name=f"att{l}", bufs=1) as ap, \
         tc.tile_pool(name=f"aps{l}", bufs=1, space="PSUM") as aps:
            g16 = [ap.tile([P, 256], F16, tag=f"g16{ct}", name=f"g16{ct}") for ct in range(2)]
            for ct in range(2):
                g_f = ap.tile([P, 256], F32, tag="gf", name="gf", bufs=2)
                nc.sync.dma_start(g_f[:], cc_out[P * ct: P * (ct + 1), :])
                nc.scalar.activation(g16[ct][:], g_f[:], AF.Copy,
                                     scale=GSC)

            # Bq = G @ Wq, Bk = G @ Wk  (fp16, channel-major halves)
            bq, bk = [], []
            for j, (wmat, blist) in enumerate(((w["wq"], bq), (w["wk"], bk))):
                for mt in range(2):
                    b_ps = aps.tile([P, 256], F32, tag="bps", name="bps")
                    for ct in range(2):
                        nc.tensor.matmul(b_ps[:],
                                         g16[ct][:, P * mt: P * (mt + 1)],
                                         wmat[ct][:],
                                         start=(ct == 0), stop=(ct == 1))
                    b_sb = ap.tile([P, 256], F16, tag=f"bsb{j}{mt}",
                                   name=f"bsb{j}{mt}")
                    nc.scalar.activation(b_sb[:], b_ps[:], AF.Copy)
                    blist.append(b_sb)

            # attention score blocks A_h = Wk_h^T (G Wq)_h  -> [128, 2, 128]
            a_ps = aps.tile([P, 256], F32, tag="aps", name="aps")
            for h in range(HEADS):
                hc, r = h // 4, h % 4
                for mt in range(2):
                    nc.tensor.matmul(
                        a_ps[32 * r: 32 * r + 32,
                             P * hc + 32 * r: P * hc + 32 * r + 32],
                        w["wk"][mt][:, 32 * h: 32 * h + 32],
                        bq[mt][:, 32 * h: 32 * h + 32],
                        start=(mt == 0), stop=(mt == 1),
                        tile_position=(0, 32 * r))

            # norms: sqq/sqk rows = diag(W^T G W) = colsum(W o (G W))
            sq_ps = aps.tile([P, 512], F32, tag="sqps", name="sqps")
            for j, (wmat, bl) in enumerate(((w["wq"], bq), (w["wk"], bk))):
                for mt in range(2):
                    e_t = ap.tile([P, 256], F16, tag="et", name="et", bufs=2)
                    nc.vector.tensor_tensor(e_t[:], wmat[mt][:], bl[mt][:],
                                            ALU.mult)
                    nc.tensor.matmul(sq_ps[0:1, 256 * j: 256 * j + 256],
                                     ones_h[:, 0:1], e_t[:],
                                     start=(mt == 0), stop=(mt == 1))
            rsq_row = ap.tile([P, 512], F16, tag="rsqr", name="rsqr")
            with nc.allow_low_precision(reason="attn norm factors"):
                nc.scalar.activation(rsq_row[0:1, :], sq_ps[0:1, :],
                                     AF.Abs_reciprocal_sqrt,
                                     bias=eps_t[0:1, :])

            # bc: broadcast rsq_q row over partitions
            bc_ps = aps.tile([P, 256], F32, tag="bcps", name="bcps")
            nc.tensor.matmul(bc_ps[:], ones_h[0:1, :], rsq_row[0:1, 0:256],
                             start=True, stop=True)
            bc_sb = ap.tile([P, 256], F32, tag="bcsb", name="bcsb")
            nc.vector.tensor_copy(bc_sb[:], bc_ps[:])
            # sqk column form [128, 2] via PE transpose of the rsq_k row
            # ([P, 2, 2] keeps each fp16 PSUM write 4-byte aligned)
            sqk_ps = aps.tile([P, 2, 2], F16, tag="skps", name="skps")
            for hc in range(2):
                nc.tensor.transpose(sqk_ps[:, hc, 0:1],
                                    rsq_row[0:1, 256 + P * hc: 256 + P * (hc + 1)],
                                    id_sb[0:1, 0:1])
            sqk_sb = ap.tile([P, 2], F32, tag="sksb", name="sksb")
            nc.vector.tensor_tensor(sqk_sb[:], sqk_ps[:, :, 0], w["resc"][:],
                                    ALU.mult)

            # z blocks + softmax -> att16 (block-diagonal, zeros elsewhere)
            att16 = ap.tile([P, 256], F16, tag="att16", name="att16")
            nc.any.memset(att16[:], 0.0)
            z_sb = ap.tile([P, 256], F32, tag="z", name="z")
            nmax = ap.tile([P, 8], F32, tag="nmax", name="nmax")
            ssum = ap.tile([P, 8], F32, tag="ssum", name="ssum")
            nc.any.memset(ssum[:], 1.0)
            esb = ap.tile([P, 256], F32, tag="esb", name="esb")
            for h in range(HEADS):
                hc, r = h // 4, h % 4
                po, fo = 32 * r, P * hc + 32 * r
                nc.vector.scalar_tensor_tensor(
                    z_sb[po:po + 32, fo:fo + 32],
                    a_ps[po:po + 32, fo:fo + 32],
                    sqk_sb[po:po + 32, hc:hc + 1],
                    bc_sb[po:po + 32, fo:fo + 32],
                    ALU.mult, ALU.mult)
                nc.vector.tensor_reduce(nmax[po:po + 32, h:h + 1],
                                        z_sb[po:po + 32, fo:fo + 32],
                                        mybir.AxisListType.X, ALU.max,
                                        negate=True)
                nc.scalar.activation(esb[po:po + 32, fo:fo + 32],
                                     z_sb[po:po + 32, fo:fo + 32], AF.Exp,
                                     bias=nmax[po:po + 32, h:h + 1],
                                     accum_out=ssum[po:po + 32, h:h + 1])
            nc.vector.reciprocal(ssum[:, 0:8], ssum[:, 0:8])
            for h in range(HEADS):
                hc, r = h // 4, h % 4
                po, fo = 32 * r, P * hc + 32 * r
                nc.vector.tensor_scalar_mul(att16[po:po + 32, fo:fo + 32],
                                            esb[po:po + 32, fo:fo + 32],
                                            ssum[po:po + 32, h:h + 1])
            dbg("att16", att16[:], (P, 256))

            wload("oy")
            # M = attn_bd^T Wproj  (so out_c = M^T v in one GEMM pass)
            for hc in range(2):
                m_ps = aps.tile([P, 256], F32, tag=f"mps{hc}", name=f"mps{hc}")
                for mt in range(2):
                    nc.tensor.matmul(m_ps[:, P * mt: P * (mt + 1)],
                                     att16[:, P * hc: P * (hc + 1)],
                                     w["wpr"][hc][mt][:],
                                     start=True, stop=True)
                nc.scalar.activation(m_sb[hc][:], m_ps[:], AF.Copy)


def _phase4(nc, tc, l, w, xa, xb, v_sb, p_acc, m_sb, rmask, dbg):
    ry0, ry1 = EXT[l]["ry"]
    # ============ phase 4: out_c = M^T v (+bpj +p), y = x + out_c + p ===
    with tc.tile_pool(name=f"oy{l}", bufs=3) as osp, \
         tc.tile_pool(name=f"oyps{l}", bufs=2, space="PSUM") as ops:
        ybase, ytot = ry0 * WG, (ry1 - ry0) * WG
        for (o, n) in _ntiles(ytot, 512):
            for mt in range(2):
                pr_ps = ops.tile([P, 512], F32, tag=f"prps{mt}", name=f"prps{mt}")
                for hc in range(2):
                    nc.tensor.matmul(pr_ps[:, :n],
                                     m_sb[hc][:, P * mt: P * (mt + 1)],
                                     v_sb[hc][:, ybase + o: ybase + o + n],
                                     start=(hc == 0), stop=(hc == 1))
                y1 = osp.tile([P, 512], F32, tag=f"y1{mt}", name=f"y1{mt}")
                nc.vector.scalar_tensor_tensor(
                    y1[:, :n], pr_ps[:, :n], w["bpj"][mt][:],
                    p_acc[mt][:, ybase + o: ybase + o + n],
                    ALU.add, ALU.add)
                nc.gpsimd.tensor_tensor(
                    xb[mt][:, ybase + o: ybase + o + n],
                    y1[:, :n],
                    xa[mt][:, ybase + o: ybase + o + n],
                    ALU.add)

    for ct in range(2):
        _mask_rows(nc, xb[ct][:], rmask, ry0, ry1)

    dbg("y", xb[0][:], (P, NB))


def _phase5(nc, tc, l, w, xa, xb, ones_h, eps_t, cinv_t, out_d, dbg):
    ry0, ry1 = EXT[l]["ry"]
    rdw0, rdw1 = EXT[l]["rdw"]
    ytot = (ry1 - ry0) * WG
    ybase = ry0 * WG
    # ============ phase 5a: LN stats + xln for the whole row range ==========
    with tc.tile_pool(name=f"lnp{l}", bufs=1) as lp:
        xln = [lp.tile([P, ytot], F16, tag=f"xln{ct}", name=f"xln{ct}") for ct in range(2)]
        rs16 = lp.tile([P, ytot], F16, tag="rs16", name="rs16")
        with tc.tile_pool(name=f"lns{l}", bufs=1) as ls, \
             tc.tile_pool(name=f"stps{l}", bufs=1, space="PSUM") as sps:
            for (o, n) in _ntiles(ytot, 512):
                ysq = [ls.tile([P, 512], F16, tag=f"ysq{ct}", name=f"ysq{ct}", bufs=2)
                       for ct in range(2)]
                for ct in range(2):
                    nc.vector.tensor_tensor(ysq[ct][:, :n],
                                            xb[ct][:, ybase + o: ybase + o + n],
                                            xb[ct][:, ybase + o: ybase + o + n],
                                            ALU.mult)
                ssum = sps.tile([P, 512], F32, tag="ssum", name="ssum", bufs=2)
                ssq = sps.tile([P, 512], F32, tag="ssq", name="ssq", bufs=2)
                for ct in range(2):
                    nc.tensor.matmul(ssum[:, :n], ones_h[:, :],
                                     xb[ct][:, ybase + o: ybase + o + n],
                                     start=(ct == 0), stop=(ct == 1))
                    nc.tensor.matmul(ssq[:, :n], ones_h[:, :],
                                     ysq[ct][:, :n],
                                     start=(ct == 0), stop=(ct == 1))
                mu2 = ls.tile([P, 512], F32, tag="mu2", name="mu2", bufs=2)
                nc.scalar.activation(mu2[:, :n], ssum[:, :n], AF.Square,
                                     scale=cinv_t[:])
                rs = ls.tile([P, 512], F32, tag="rs", name="rs", bufs=2)
                nc.vector.scalar_tensor_tensor(rs[:, :n], ssq[:, :n],
                                               1.0 / C, mu2[:, :n],
                                               ALU.mult, ALU.subtract)
                nc.scalar.activation(rs16[:, o:o + n], rs[:, :n],
                                     AF.Abs_reciprocal_sqrt, bias=eps_t[:])
                for ct in range(2):
                    d = ls.tile([P, 512], F16, tag=f"d{ct}", name=f"d{ct}", bufs=2)
                    nc.vector.scalar_tensor_tensor(
                        d[:, :n], ssum[:, :n], -1.0 / C,
                        xb[ct][:, ybase + o: ybase + o + n],
                        ALU.mult, ALU.add)
                    nc.vector.tensor_tensor(xln[ct][:, o:o + n], d[:, :n],
                                            rs16[:, o:o + n], ALU.mult)
        dbg("xln", xln[0][:], (P, ytot))

        # ============ phase 5b: FF (chunked over rows) ======================
        with tc.tile_pool(name=f"ff{l}", bufs=1) as fp, \
             tc.tile_pool(name=f"ffs{l}", bufs=1) as fs, \
             tc.tile_pool(name=f"ffps{l}", bufs=3, space="PSUM") as fps:
            for (co, cn) in _ntiles(ry1 - ry0, FF_CH):
                a, b = ry0 + co, ry0 + co + cn
                w0, w1 = max(a - 1, ry0), min(b + 1, ry1)
                c0, c1 = max(a, rdw0), min(b, rdw1)
                wlen = (w1 - w0) * WG
                xo = (w0 - ry0) * WG
                # --- ff1 + gelu -> t1 ---
                t1 = [fs.tile([P, wlen + 2], F16, tag=f"t1{mt}", name=f"t1{mt}") for mt in range(8)]
                t1s = [fs.tile([P, wlen + 2], F16, tag=f"t1s{mt}", name=f"t1s{mt}")
                       if FFDW_ENG[mt] != "pe" else None for mt in range(8)]
                for mt in range(8):
                    for (o, n) in _ntiles(wlen, 512):
                        f1_ps = fps.tile([P, 512], F32, tag="ffps", name="ffps")
                        for kt in range(2):
                            nc.tensor.matmul(f1_ps[:, :n], w["f1t"][kt][mt][:],
                                             xln[kt][:, xo + o: xo + o + n],
                                             start=(kt == 0), stop=(kt == 1))
                        nc.scalar.activation(t1[mt][:, o:o + n], f1_ps[:, :n],
                                             AF.Gelu, bias=w["b1"][mt][:])
                    _zero_guards(nc, t1[mt][:, 0:wlen], 0, w1 - w0)
                    nc.any.memset(t1[mt][:, wlen:wlen + 2], 0.0)
                    if FFDW_ENG[mt] != "pe":
                        _shift1(nc, t1s[mt][:], t1[mt][:, 0:wlen], wlen)
                # --- ffdw (4 lanes) + gelu -> t2 ---
                t2 = [fs.tile([P, (c1 - c0) * WG], F16, tag=f"t2{mt}", name=f"t2{mt}")
                      for mt in range(8)]
                for mt in range(8):
                    eng = FFDW_ENG[mt]
                    if eng == "pe":
                        base = (c0 - w0) * WG
                        for (o, n) in _ntiles((c1 - c0) * WG - 1, 512):
                            dw_ps = fps.tile([P, 512], F32, tag="dwps", name="dwps", bufs=2)
                            _conv9_pe(nc, dw_ps[:, :n], w["dwdiag"][mt],
                                      t1[mt][:], c0 - w0, o + 1, n)
                            nc.scalar.activation(t2[mt][:, 1 + o:1 + o + n],
                                                 dw_ps[:, :n], AF.Gelu)
                        nc.any.memset(t2[mt][:, 0:1], 0.0)
                    else:
                        acc = _conv9(nc, fs, f"dw{mt}", None, t1[mt][:, 0:wlen],
                                     t1s[mt][:], w["dww"][mt], c0 - w0, c1 - w0,
                                     mul_eng=("act" if eng == "act" else "dve"),
                                     add_eng=("pool" if eng == "pool" else "dve"))
                        nc.scalar.activation(t2[mt][:], acc, AF.Gelu)
                # --- ff3 + residual -> x2 (= xa), or final output ---
                for mt in range(2):
                    for (o, n) in _ntiles((c1 - c0) * WG, 512):
                        f3_ps = fps.tile([P, 512], F32, tag="f3ps", name="f3ps", bufs=2)
                        for kt in range(8):
                            nc.tensor.matmul(f3_ps[:, :n], w["f3t"][kt][mt][:],
                                             t2[kt][:, o:o + n],
                                             start=(kt == 0), stop=(kt == 7))
                        nc.vector.tensor_tensor(
                            xa[mt][:, c0 * WG + o: c0 * WG + o + n],
                            f3_ps[:, :n],
                            xb[mt][:, c0 * WG + o: c0 * WG + o + n], ALU.add)
                for mt in range(2):
                    _zero_guards(nc, xa[mt][:], c0, c1)

    if out_d is not None:
        with tc.tile_pool(name="outp", bufs=1) as op_:
            for ct in range(2):
                o32 = op_.tile([P, 32 * WG], F32, tag=f"o32{ct}", name=f"o32{ct}")
                nc.scalar.activation(o32[:], xa[ct][:, 6 * WG: 38 * WG], AF.Copy)
                nc.sync.dma_start(out_d.ap()[ct], _r3(o32[:])[:, :, 1:129])
    else:
        dbg("x2", xa[0][:], (P, NB))


# ======================== host side =========================================

_CACHE = {}


def _prep_shards(x, illu_fea, Wq, Wk, Wv, rescale, Wproj, bproj, pos1, pos2,
                 ln_g, ln_b, ff1, ffdw, ff3):
    f16 = ml_dtypes.float16 if hasattr(ml_dtypes, "float16") else np.float16

    def pad_spatial(t):  # (B,C,H,W) -> per-core [2, 128, HB, WG]
        out = []
        for core in range(N_CORES):
            bb, ss = core // 4, core % 4
            r0 = 32 * ss - HALO
            buf = np.zeros((C, HB, WG), np.float32)
            lo, hi = max(r0, 0), min(r0 + HB, H)
            buf[:, lo - r0: hi - r0, 1:129] = t[bb, :, lo:hi, :]
            out.append(buf.reshape(2, P, HB, WG))
        return out

    xs = pad_spatial(np.asarray(x, np.float32))
    ils = pad_spatial(np.asarray(illu_fea, np.float32))

    # channel-major attention weights [ct][c_local, m]
    wq = np.stack([Wq[l].reshape(2, P, 256) for l in range(LYR)])
    wk = np.stack([Wk[l].reshape(2, P, 256) for l in range(LYR)])
    wv = np.stack([Wv[l].reshape(2, P, 2, P).transpose(0, 2, 1, 3)
                   for l in range(LYR)])
    wpr = np.stack([Wproj[l].reshape(2, P, 2, P).transpose(0, 2, 1, 3)
                    for l in range(LYR)])
    # fold ln_g into ff1 (per input channel)
    f1 = np.stack([(ff1[l, :, :, 0, 0] * ln_g[l][None, :]).T
                   .reshape(2, P, 8, P).transpose(0, 2, 1, 3)
                   for l in range(LYR)])
    f3 = np.stack([ff3[l, :, :, 0, 0].T.reshape(8, P, 2, P).transpose(0, 2, 1, 3)
                   for l in range(LYR)])
    posw = np.stack([np.stack([p[l, :, 0].reshape(C, 9).reshape(2, P, 9)
                               for p in (pos1, pos2)]) for l in range(LYR)])
    dww = np.stack([ffdw[l, :, 0].reshape(FC, 9).reshape(8, P, 9)
                    for l in range(LYR)])
    dwdg = np.zeros((LYR, 8, 9, P, P), np.float32)
    psdg = np.zeros((LYR, 2, 2, 9, P, P), np.float32)
    ii = np.arange(P)
    for l in range(LYR):
        for mt in range(8):
            for t in range(9):
                dwdg[l, mt, t, ii, ii] = dww[l, mt, :, t]
        for cv in range(2):
            for ct in range(2):
                for t in range(9):
                    psdg[l, cv, ct, t, ii, ii] = posw[l, cv, ct, :, t]
    b1 = np.stack([(ff1[l, :, :, 0, 0] @ ln_b[l]).reshape(8, P, 1)
                   for l in range(LYR)])
    bpj = np.asarray(bproj, np.float32).reshape(LYR, 2, P, 1)
    # resc in sqk column layout: resc_col[p, hc] = rescale[head of 128*hc+p]
    resc = np.zeros((LYR, P, 2), np.float32)
    for l in range(LYR):
        for hc in range(2):
            for p in range(P):
                resc[l, p, hc] = rescale[l, (P * hc + p) // D, 0, 0]

    const = {
        "wq": wq.astype(f16), "wk": wk.astype(f16),
        "wv": wv.astype(f16), "wpr": wpr.astype(f16),
        "f1t": f1.astype(f16), "f3t": f3.astype(f16),
        "posw": posw.astype(np.float32), "dww": dww.astype(np.float32),
        "dwdg": dwdg.astype(f16), "psdg": psdg.astype(f16),
        "b1": b1.astype(np.float32), "bpj": bpj,
        "resc": resc.astype(np.float32),
        "id128": np.eye(P, dtype=np.float32).astype(f16),
    }
    in_maps = []
    for core in range(N_CORES):
        m = dict(const)
        ss = core % 4
        r0 = 32 * ss - HALO
        rmv = np.zeros((P, HB), np.float32)
        for r in range(HB):
            rmv[:, r] = 1.0 if 0 <= r0 + r < H else 0.0
        m["rmask"] = rmv.astype(np.float32)
        m["xh"] = xs[core].reshape(2, P, NB).astype(f16)
        m["il"] = ils[core].reshape(2, P, NB).astype(f16)
        in_maps.append(m)
    return in_maps


def _get_nc():
    if "nc" not in _CACHE:
        _CACHE["nc"], _CACHE["dbg"] = build()
    return _CACHE["nc"]


def run(in_maps, trace=False):
    nc = _get_nc()
    return bass_utils.run_bass_kernel_spmd(
        nc, in_maps, core_ids=list(range(N_CORES)), trace=trace)


def kernel(**inputs):
    in_maps = _prep_shards(**{k: np.asarray(v) for k, v in inputs.items()})
    res = run(in_maps)
    out = np.zeros((B, C, H, W), np.float32)
    for core in range(N_CORES):
        bb, ss = core // 4, core % 4
        o = res.results[core]["out"]  # [2, 128, 32, 128]
        out[bb, :, 32 * ss: 32 * ss + 32, :] = o.reshape(C, 32, W)
    return out
